# revision 5
# baseline (speedup 1.0000x reference)
"""Trainium2 Bass kernel for nn_BinaryLayer: out = sign(x @ sign(W)).

x: [8192, 2048] f32, W: [2048, 2048] f32, out: [8192, 2048] f32 (values in {-1,0,1}).

Strategy: data-parallel batch shard across 8 cores (1024 rows each), W replicated.
Each core:
  - loads W in [128, 512] column chunks (per (k-tile, n-chunk) so first-n-chunk
    matmuls start as soon as their chunks land), binarizes on ScalarE (Sign),
  - loads x^T k-tiles [128, BS] f32 (host pre-transposes each shard so the
    contraction dim lands on partitions; pure layout prep),
  - matmuls accumulate over 16 k-tiles into PSUM banks [128, 512],
  - Sign activation PSUM -> SBUF f32, DMA to out.

Loop order is n-chunk outer / k middle / m inner so the PE consumes W and x
k-tiles as they stream from HBM.

MODE:
  "hilo2" - 2-pass bf16 hi/lo: hi = bf16(x), lo = bf16(x - hi) on VectorE; both
            passes accumulate into the same PSUM bank. Products are exact
            (weights are +-1), so only the hi+lo representation error
            (~2^-18 relative) plus fp32 PSUM accumulation order remains ->
            near-fp32-exact. PE ~218us/core.
  "f32r1" - 1-pass float32r (FP22 truncation on PE read) for both operands;
            W binarized in place as f32 (+-1.0 is fp22-exact). Half the PE
            time (~109us/core), ~1e-2 L2 rel err from the 2^-14 truncation
            of x. Batch is processed in two halves so W f32 (128KB/part) +
            x half (32KB/part) fit SBUF.
"""

import numpy as np

B, D_IN, D_OUT = 8192, 2048, 2048
N_CORES = 8
BS = B // N_CORES  # 1024 batch rows per core
P = 128
KT = D_IN // P  # 16 k-tiles
NCH = 512  # psum bank width (f32)
NT = D_OUT // NCH  # 4 n-chunks

MODE = "hilo2"

_CACHE: dict = {}


def build_bass(mode: str = MODE):
    import concourse.mybir as mybir
    import concourse.tile as tile
    from concourse import bacc
    from contextlib import ExitStack

    f32 = mybir.dt.float32
    bf16 = mybir.dt.bfloat16
    f32r = mybir.dt.float32r
    Sign = mybir.ActivationFunctionType.Sign

    # Bacc (not plain Bass): its finalize() runs move_matmul_waits_to_ldweights
    # + generate_event_semaphores, which legalize multi-wait instructions for
    # walrus (each non-event instruction may carry at most one sync wait).
    nc = bacc.Bacc()
    xT = nc.declare_dram_parameter("xT", [D_IN, BS], f32, isOutput=False)
    w = nc.declare_dram_parameter("w", [D_IN, D_OUT], f32, isOutput=False)
    out = nc.declare_dram_parameter("out", [BS, D_OUT], f32, isOutput=True)

    with ExitStack() as ctx:
        tc = ctx.enter_context(tile.TileContext(nc))
        res_pool = ctx.enter_context(tc.tile_pool(name="resident", bufs=1))
        xstage = ctx.enter_context(tc.tile_pool(name="xstage", bufs=2))
        psum_pool = ctx.enter_context(tc.tile_pool(name="psum", bufs=8, space="PSUM"))
        ostage = ctx.enter_context(tc.tile_pool(name="ostage", bufs=4))

        wdt = bf16 if mode == "hilo2" else f32
        wbin = [
            [
                res_pool.tile([P, NCH], wdt, tag=f"wb{k}_{n}", name=f"wb{k}_{n}")
                for n in range(NT)
            ]
            for k in range(KT)
        ]

        def load_w_chunk(k, n):
            if mode == "hilo2":
                w32 = xstage.tile([P, NCH], f32, tag="w32", name="w32", bufs=3)
                nc.sync.dma_start(
                    w32[:], w[k * P : (k + 1) * P, n * NCH : (n + 1) * NCH]
                )
                nc.scalar.activation(wbin[k][n][:], w32[:], Sign)
            else:
                # f32r path: load into the resident tile and binarize in place.
                nc.sync.dma_start(
                    wbin[k][n][:], w[k * P : (k + 1) * P, n * NCH : (n + 1) * NCH]
                )
                nc.scalar.activation(wbin[k][n][:], wbin[k][n][:], Sign)

        if mode == "hilo2":
            MT = BS // P  # 8 m-tiles
            xhi = [
                res_pool.tile([P, BS], bf16, tag=f"xhi{k}", name=f"xhi{k}")
                for k in range(KT)
            ]
            xlo = [
                res_pool.tile([P, BS], bf16, tag=f"xlo{k}", name=f"xlo{k}")
                for k in range(KT)
            ]

            # Stream: x k-tiles + the n=0 W column chunks first, then the rest.
            for k in range(KT):
                x32 = xstage.tile([P, BS], f32, tag="x32", name="x32")
                nc.sync.dma_start(x32[:], xT[k * P : (k + 1) * P, :])
                nc.vector.tensor_copy(xhi[k][:], x32[:])
                nc.vector.tensor_sub(xlo[k][:], x32[:], xhi[k][:])
                load_w_chunk(k, 0)
            for n in range(1, NT):
                for k in range(KT):
                    load_w_chunk(k, n)

            for n in range(NT):
                psums = [
                    psum_pool.tile([P, NCH], f32, tag="ps", name="ps")
                    for _ in range(MT)
                ]
                for k in range(KT):
                    for pi, src in enumerate((xhi, xlo)):
                        for m in range(MT):
                            nc.tensor.matmul(
                                psums[m][:],
                                src[k][:, m * P : (m + 1) * P],
                                wbin[k][n][:],
                                start=(k == 0 and pi == 0),
                                stop=(k == KT - 1 and pi == 1),
                            )
                for m in range(MT):
                    ot = ostage.tile([P, NCH], f32, tag="ot", name="ot")
                    nc.scalar.activation(ot[:], psums[m][:], Sign)
                    nc.sync.dma_start(
                        out[m * P : (m + 1) * P, n * NCH : (n + 1) * NCH], ot[:]
                    )

        elif mode == "f32r1":
            NBH = 2  # batch halves (SBUF: W f32 128KB/part + x half 32KB/part)
            BS2 = BS // NBH  # 512
            MT2 = BS2 // P  # 4 m-tiles per half
            xres = [
                res_pool.tile([P, BS2], f32, tag=f"xr{k}", name=f"xr{k}")
                for k in range(KT)
            ]

            for bh in range(NBH):
                for k in range(KT):
                    if bh == 0:
                        # First half: interleave x with the n=0 W chunks.
                        nc.sync.dma_start(
                            xres[k][:], xT[k * P : (k + 1) * P, 0:BS2]
                        )
                        load_w_chunk(k, 0)
                    else:
                        # Second half: re-fill the same tiles (bufs=1 makes the
                        # DMA wait for the first half's last consumer).
                        nc.sync.dma_start(
                            xres[k][:],
                            xT[k * P : (k + 1) * P, bh * BS2 : (bh + 1) * BS2],
                        )
                if bh == 0:
                    for n in range(1, NT):
                        for k in range(KT):
                            load_w_chunk(k, n)

                for n in range(NT):
                    psums = [
                        psum_pool.tile([P, NCH], f32, tag="ps", name="ps")
                        for _ in range(MT2)
                    ]
                    for k in range(KT):
                        for m in range(MT2):
                            nc.tensor.matmul(
                                psums[m][:],
                                xres[k][:, m * P : (m + 1) * P].bitcast(f32r),
                                wbin[k][n][:].bitcast(f32r),
                                start=(k == 0),
                                stop=(k == KT - 1),
                            )
                    for m in range(MT2):
                        ot = ostage.tile([P, NCH], f32, tag="ot", name="ot")
                        nc.scalar.activation(ot[:], psums[m][:], Sign)
                        nc.sync.dma_start(
                            out[
                                bh * BS2 + m * P : bh * BS2 + (m + 1) * P,
                                n * NCH : (n + 1) * NCH,
                            ],
                            ot[:],
                        )
        else:
            raise ValueError(mode)

    nc.finalize()
    return nc


def _shard_inputs(x: np.ndarray, kernel: np.ndarray):
    """Per-core input maps: batch-shard x (pre-transposed layout), replicate W."""
    in_maps = []
    for i in range(N_CORES):
        xs = np.ascontiguousarray(x[i * BS : (i + 1) * BS, :].T)
        in_maps.append({"xT": xs, "w": kernel})
    return in_maps


def run_on_cores(x: np.ndarray, kernel: np.ndarray, mode: str = MODE, **run_kwargs):
    """Compile (cached) and run the SPMD kernel; returns (full_out, BassKernelResults)."""
    from concourse.bass_utils import run_bass_kernel_spmd

    key = ("nc", mode)
    if key not in _CACHE:
        _CACHE[key] = build_bass(mode)
    nc = _CACHE[key]

    in_maps = _shard_inputs(x, kernel)
    res = run_bass_kernel_spmd(nc, in_maps, list(range(N_CORES)), **run_kwargs)
    out = np.concatenate([res.results[i]["out"] for i in range(N_CORES)], axis=0)
    return out, res


def kernel(x: np.ndarray, kernel: np.ndarray) -> np.ndarray:
    assert x.shape == (B, D_IN) and kernel.shape == (D_IN, D_OUT)
    out, _ = run_on_cores(
        np.asarray(x, dtype=np.float32), np.asarray(kernel, dtype=np.float32)
    )
    return out.astype(np.float32)
